# revision 52
# baseline (speedup 1.0000x reference)
"""Causal multi-head self-attention on 8 Trainium2 NeuronCores.

Sharding: core = (batch b, head-group g).  B=4 batches x 2 groups of 8 heads
= 8 cores.  Each core computes Q/K/V projections for its 8 heads, causal
attention, and a partial output projection (row-shard of WO); the host sums
the two partials per batch (the tensor-parallel all-reduce, done at gather).

v7 (fused): v6 ran projections as a separate phase (PE 97% busy, ACT idle),
then an exp-paced attention phase whose PE micro-gaps made the HAM throttle
oscillate K=4/K=8 for ~74us (half-clock PE).  v7 fuses the two: only
s-block 0's projections run up front; s-block sb+1's projection matmuls are
split into half-chain pieces and pumped as PE filler between attention chunk
groups of q-block sb.  The PE queue then always has ready work during exp
stalls, which both fills the gaps and keeps HAM at full clock.

Attention machinery (from v6, unchanged): score tiles for a head PAIR in one
2-bank PSUM tile [128, 2, 512] so each k-chunk gets ONE exp covering both
heads; causal stripe mask as a 0/1 multiply on idle GpSimd; diagonal chunks
trimmed to [cs:]; scores/attnV pipeline with a 2-chunk lag running
continuously across pair and q-block boundaries; previous q-block's
normalization + output projection sliced into pieces pumped between attnV
groups.  PSUM: score 2 tags x 2 banks + ovpo 2 x 1 + aux (proj/rb/po)
2 x 1 = 8 banks.
"""

import os
import numpy as np
import ml_dtypes

B, S, D = 4, 2048, 1024
H_TOTAL, DK = 16, 64
G = 2          # head groups (cores per batch)
HG = 8         # heads per core
DG = 512       # head dims per core
CO = 8         # contraction chunks of 128 over D
SBLK = 4       # 512-wide s blocks
QB = 4         # 512-wide q blocks
NEG = -1e9

_BUILD_CACHE = {}


def _build():
    if "nc" in _BUILD_CACHE:
        return _BUILD_CACHE["nc"]

    from collections import deque

    import concourse.bacc as bacc
    import concourse.mybir as mybir
    import concourse.tile as tile
    from concourse.tile_rust import add_dep_helper

    f32 = mybir.dt.float32
    bf16 = mybir.dt.bfloat16
    AF = mybir.ActivationFunctionType
    MULT = mybir.AluOpType.mult

    nc = bacc.Bacc("TRN2", target_bir_lowering=False)
    xT_d = nc.dram_tensor("xT", [D, S], bf16, kind="ExternalInput")
    wq_d = nc.dram_tensor("wqT", [D, DG], bf16, kind="ExternalInput")
    wk_d = nc.dram_tensor("wkT", [D, DG], bf16, kind="ExternalInput")
    wv_d = nc.dram_tensor("wvT", [D, DG], bf16, kind="ExternalInput")
    wo_d = nc.dram_tensor("woT", [DG, D], bf16, kind="ExternalInput")
    mask_d = nc.dram_tensor("mask", [128, 256], bf16, kind="ExternalInput")
    ones_d = nc.dram_tensor("onesb", [128, 128], bf16, kind="ExternalInput")
    ones32b_d = nc.dram_tensor("ones32", [128, 64], bf16, kind="ExternalInput")
    out_d = nc.dram_tensor("out", [S, D], bf16, kind="ExternalOutput")

    with tile.TileContext(nc) as tc:
        with (
            tc.tile_pool(name="sb", bufs=1) as pp,
            tc.tile_pool(name="ps", bufs=1, space="PSUM") as ps,
        ):
            QT = pp.tile([128, 4, S], bf16, tag="QT")
            KT = pp.tile([128, 4, S], bf16, tag="KT")
            V = pp.tile([128, 16, HG, DK + 1], bf16, tag="V")
            AT = pp.tile([128, 4, S], bf16, tag="AT")
            wo = pp.tile([128, 4, D], bf16, tag="wo")
            maskb = pp.tile([128, 2, 128], bf16, tag="maskb")
            onesb = pp.tile([128, 128], bf16, tag="onesb")
            ones_r = pp.tile([128, 64], bf16, tag="ones_r")
            wv = pp.tile([128, CO, DG], bf16, tag="wv")
            wq = pp.tile([128, CO, DG], bf16, tag="wq")
            wk = pp.tile([128, CO, DG], bf16, tag="wk")

            # Input DMA is ~200-230 GB/s aggregate no matter how many queues
            # carry it, so everything rides one queue, ordered so the bytes
            # that gate qb0's first scores (xt0 + wq + wk) land first.
            nc.sync.dma_start(
                maskb[:], mask_d[:, :].rearrange("p (h c) -> p h c", h=2)
            )
            nc.sync.dma_start(onesb[:], ones_d[:, :])
            nc.sync.dma_start(ones_r[:], ones32b_d[:, :])
            # ones column of V (bf16 broadcast copy)
            nc.vector.tensor_copy(
                V[:, :, :, DK : DK + 1],
                onesb[:, 0:128].rearrange("p (so h) -> p so h", so=16)[:, :, :, None],
            )

            # ---------------- PE emission-order machinery ----------------
            pe_prev = [None]  # last instr of the previous PE group

            def pe_group(insts):
                if not insts:
                    return
                if pe_prev[0] is not None:
                    add_dep_helper(
                        insts[0].ins, pe_prev[0].ins, sync=False,
                        reason="pe group order",
                    )
                for a, b in zip(insts[1:], insts):
                    add_dep_helper(a.ins, b.ins, sync=False, reason="pe chain")
                pe_prev[0] = insts[-1]

            # ---------------- projection pieces ----------------
            # Each s-block's projections are 24 half-chain pieces (~0.85us of
            # PE each): pq/pk per 128-dim group and pv per 128-row s chunk,
            # each an 8-matmul PSUM chain split in two.
            xt_tiles = {}

            def load_xt(sb, split):
                xt = pp.tile([128, CO, 512], bf16, tag="xt", bufs=2, name=f"xt{sb}")
                ssl = slice(sb * 512, (sb + 1) * 512)
                if split:
                    nc.sync.dma_start(
                        xt[:, 0:4, :],
                        xT_d[0:512, ssl].rearrange("(co ci) s -> ci co s", ci=128),
                    )
                else:
                    nc.sync.dma_start(
                        xt, xT_d[:, ssl].rearrange("(co ci) s -> ci co s", ci=128)
                    )
                xt_tiles[sb] = xt
                return xt

            def load_xt_rest(sb):
                ssl = slice(sb * 512, (sb + 1) * 512)
                nc.sync.dma_start(
                    xt_tiles[sb][:, 4:8, :],
                    xT_d[512:1024, ssl].rearrange("(co ci) s -> ci co s", ci=128),
                )

            def mk_proj_pieces(sb, kind, idx):
                xt = xt_tiles[sb]
                ssl = slice(sb * 512, (sb + 1) * 512)
                holder = [None]

                def half(h):
                    def piece():
                        if holder[0] is None:
                            holder[0] = ps.tile(
                                [128, 512], f32, tag="aux", bufs=2,
                                name=f"p{kind}{sb}_{idx}",
                            )
                        pt = holder[0]
                        dsl = slice(idx * 128, (idx + 1) * 128)
                        grp = []
                        for co in range(4 * h, 4 * h + 4):
                            if kind == "v":
                                m = nc.tensor.matmul(
                                    pt, xt[:, co, dsl], wv[:, co, :],
                                    start=(co == 0), stop=(co == CO - 1),
                                )
                            elif kind == "q":
                                m = nc.tensor.matmul(
                                    pt, wq[:, co, dsl], xt[:, co, :],
                                    start=(co == 0), stop=(co == CO - 1),
                                )
                            else:
                                m = nc.tensor.matmul(
                                    pt, wk[:, co, dsl], xt[:, co, :],
                                    start=(co == 0), stop=(co == CO - 1),
                                )
                            grp.append(m)
                        pe_group(grp)
                        if h == 1:
                            if kind == "v":
                                nc.vector.tensor_copy(
                                    V[:, sb * 4 + idx, :, 0:DK],
                                    pt[:].rearrange("p (h d) -> p h d", h=HG),
                                )
                            elif kind == "q":
                                nc.vector.tensor_copy(QT[:, idx, ssl], pt[:])
                            else:
                                nc.vector.tensor_copy(KT[:, idx, ssl], pt[:])
                    return piece
                return [half(0), half(1)]

            proj_q = deque()

            def enqueue_proj(sb):
                load_xt(sb, split=False)
                for do in range(4):
                    proj_q.extend(mk_proj_pieces(sb, "q", do))
                    proj_q.extend(mk_proj_pieces(sb, "k", do))
                for so in range(4):
                    proj_q.extend(mk_proj_pieces(sb, "v", so))

            # ---------------- s-block 0 projections ----------------
            load_xt(0, split=True)
            nc.sync.dma_start(
                wq[:, 0:4, :], wq_d[0:512, :].rearrange("(co ci) d -> ci co d", ci=128)
            )
            nc.sync.dma_start(
                wq[:, 4:8, :], wq_d[512:1024, :].rearrange("(co ci) d -> ci co d", ci=128)
            )
            load_xt_rest(0)
            nc.sync.dma_start(
                wk[:, 0:4, :], wk_d[0:512, :].rearrange("(co ci) d -> ci co d", ci=128)
            )
            nc.sync.dma_start(
                wk[:, 4:8, :], wk_d[512:1024, :].rearrange("(co ci) d -> ci co d", ci=128)
            )
            nc.sync.dma_start(
                wv[:, 0:4, :], wv_d[0:512, :].rearrange("(co ci) d -> ci co d", ci=128)
            )
            nc.sync.dma_start(
                wv[:, 4:8, :], wv_d[512:1024, :].rearrange("(co ci) d -> ci co d", ci=128)
            )

            # Only the first head pair's Q/K runs up front: qb0's first
            # scores need just pq/pk(do0).  V and the remaining Q/K become
            # filler pumped under qb0's exp stream (V loads after wq/wk on
            # the wire, so V chains would head-of-line block scores if
            # emitted here).
            for p in mk_proj_pieces(0, "q", 0):
                p()
            for p in mk_proj_pieces(0, "k", 0):
                p()
            proj_q.extend(mk_proj_pieces(0, "q", 1))
            proj_q.extend(mk_proj_pieces(0, "k", 1))
            for so in range(4):
                proj_q.extend(mk_proj_pieces(0, "v", so))
            for do in range(2, 4):
                proj_q.extend(mk_proj_pieces(0, "q", do))
                proj_q.extend(mk_proj_pieces(0, "k", do))

            # xt(sb1) must be on the wire before qb0's filler pieces need
            # it; wo is only read from qb1 on, so it loads last.
            enqueue_proj(1)
            nc.sync.dma_start(wo, wo_d[:, :].rearrange("(io ip) j -> ip io j", ip=128))

            # ---------------- attention + output ----------------
            # two-chunk-lag pipeline state, continuous across pairs/qbs
            pend = []              # [(emit_fn, items)] for previous chunks
            fin_pieces = []        # deferred finalize piece queue
            after_registry = {}    # (qb, pair) -> enqueue-finalize callback
            deferred_po = []       # qb0/qb1 output projections, held for qb3
                                   # (the exp-heaviest q-block needs the most
                                   # PE filler; qb1/qb2 have proj filler)

            def pump_fin(n=1):
                for _ in range(n):
                    if not fin_pieces:
                        return
                    fin_pieces.pop(0)()

            def pump_proj(n):
                for _ in range(n):
                    if not proj_q:
                        return
                    proj_q.popleft()()

            def pump_any(allowance):
                # spacer: ONLY dependency-light work (proj chains, deferred
                # po) may fill between attention groups.  fin pieces (recip/
                # rb_norm) wait on just-issued DVE work and head-of-line
                # block the PE FIFO if pumped faster than 1/slot.
                if allowance[0] <= 0 or not proj_q:
                    return
                proj_q.popleft()()
                allowance[0] -= 1

            def flush_one(allowance=None):
                if pend:
                    emit_fn, items = pend.pop(0)
                    emit_fn(items, allowance)

            def flush_pend():
                while pend:
                    flush_one()

            PROJ_BUDGET = {0: 6, 1: 2, 2: 1, 3: 2}

            for qb in range(QB):
                if 0 < qb < 3:
                    enqueue_proj(qb + 1)
                if qb == 3:
                    proj_q.extend(deferred_po)
                    deferred_po.clear()
                qsl = slice(qb * 512, (qb + 1) * 512)
                nkb = 4 * qb + 4
                # Denominator tiles.  Rows are 32-aligned (partition shifts
                # from PSUM row 64 must be multiples of 32); garbage lanes
                # are fine, the reciprocal output is only read at the real
                # rows.  For qb<3 one a/b tile pair per q-block (2
                # reciprocals); for the last q-block one tile PER HEAD PAIR
                # so each pair's normalization runs as soon as that pair
                # finishes, keeping the epilogue off the critical tail.
                if qb < 3:
                    sums_ab = (
                        pp.tile([128, 512], f32, tag="sums", bufs=3, name=f"sums_a{qb}"),
                        pp.tile([128, 512], f32, tag="sums", bufs=3, name=f"sums_b{qb}"),
                        pp.tile([128, 512], bf16, tag="srec", bufs=3, name=f"srec_a{qb}"),
                        pp.tile([128, 512], bf16, tag="srec", bufs=3, name=f"srec_b{qb}"),
                    )
                for pair in range(4):
                    heads = (2 * pair, 2 * pair + 1)
                    if qb < 3:
                        sums_p = None
                    else:
                        sums_p = pp.tile(
                            [128, 512], f32, tag="sums", bufs=3, name=f"sums{qb}p{pair}"
                        )
                    ovs = {}

                    def make_emit_avs(heads=heads, ovs=ovs, nkb=nkb,
                                      sums_p=sums_p, qsl=qsl, qb=qb,
                                      pair=pair,
                                      sums_ab=(sums_ab if qb < 3 else None),
                                      last=False):
                        def emit_avs(items, allowance=None):
                            if not ovs:
                                for h in heads:
                                    ovs[h] = ps.tile(
                                        [DK + 1, 512], f32, tag="ovpo", bufs=2,
                                        name=f"ov{h}",
                                    )
                            # kb-major with a proj spacer between kbs: each
                            # kb's matmuls wait on their own exp, and ready
                            # filler sits between the waits instead of
                            # behind them in the PE FIFO.
                            for (pkb, pcs, pet) in items:
                                grp = []
                                for h in heads:
                                    h2 = h % 2
                                    grp.append(nc.tensor.matmul(
                                        ovs[h][:, pcs:], V[:, pkb, h, :],
                                        pet[:, h2, pcs:],
                                        start=(pkb == 0), stop=(pkb == nkb - 1),
                                    ))
                                pe_group(grp)
                                if allowance is not None:
                                    pump_any(allowance)
                            if last:
                                for h in heads:
                                    ov = ovs[h]
                                    if sums_p is None:
                                        dst = sums_ab[h % 2]
                                        r32 = pair * 32
                                    else:
                                        dst = sums_p
                                        r32 = (h % 2) * 32
                                    nc.vector.tensor_copy(
                                        dst[r32 : r32 + 1, :],
                                        ov[DK : DK + 1, :],
                                    )
                                for h in heads:
                                    base = 64 * (h % 2)
                                    nc.vector.tensor_copy(
                                        AT[base : base + 64, h // 2, qsl],
                                        ovs[h][0:DK, :],
                                    )
                                cb = after_registry.pop((qb, pair), None)
                                if cb is not None:
                                    cb()
                        return emit_avs

                    # chunks of exactly 2 kbs: the sp PSUM pool holds 2
                    # tiles, so a 3-kb chunk's last scores would wait on
                    # that same chunk's first exp (intra-chunk stall); with
                    # 2-kb chunks the scores lag aligns to chunk boundaries.
                    chunks = [[2 * i, 2 * i + 1] for i in range(nkb // 2)]
                    for ci, chunk in enumerate(chunks):
                        # filler ahead of the scores group: scores(c) wait
                        # on exp(c-1) freeing their sp tiles, and anything
                        # emitted after them in the PE FIFO would stall too
                        pump_fin(1)
                        allowance = [PROJ_BUDGET[qb]]
                        pump_any(allowance)
                        items = []
                        sc_grp = []
                        for kb in chunk:
                            ksl = slice(kb * 128, (kb + 1) * 128)
                            d = kb - 4 * qb
                            cs = 128 * d if d > 0 else 0
                            sp = ps.tile(
                                [128, 2, 512], f32, tag="score", bufs=2,
                                name=f"sp{pair}q{qb}k{kb}",
                            )
                            for h in heads:
                                h2 = h % 2
                                base = 64 * h2
                                psl = slice(base, base + 64)
                                sc_grp.append(nc.tensor.matmul(
                                    sp[:, h2, cs:], KT[psl, pair, ksl],
                                    QT[psl, pair, qb * 512 + cs : (qb + 1) * 512],
                                    start=True, stop=True,
                                ))
                            et = pp.tile(
                                [128, 2, 512], bf16, tag="et", bufs=16,
                                name=f"et{pair}q{qb}k{kb}",
                            )
                            nc.scalar.activation(
                                et[:, :, cs:], sp[:, :, cs:], AF.Exp, scale=0.125
                            )
                            if d >= 0:
                                # causal stripe: zero the disallowed
                                # upper triangle on idle GpSimd
                                nc.gpsimd.tensor_tensor(
                                    et[:, :, cs : cs + 128],
                                    et[:, :, cs : cs + 128],
                                    maskb[:, :, :],
                                    MULT,
                                )
                            items.append((kb, cs, et))
                        pe_group(sc_grp)
                        if len(pend) >= 3:
                            flush_one(allowance)
                        while allowance[0] > 0 and proj_q:
                            pump_any(allowance)
                        pend.append(
                            (make_emit_avs(last=(ci == len(chunks) - 1)), items)
                        )

                    if qb < 3:
                        continue
                    srec_p = pp.tile(
                        [128, 512], bf16, tag="srec", bufs=3, name=f"srec{qb}p{pair}"
                    )
                    srf_p = pp.tile(
                        [128, 512], f32, tag="srf", bufs=2, name=f"srf{qb}p{pair}"
                    )

                    def make_recip(half, sums=sums_p, srf=srf_p):
                        def recip():
                            h0, h1 = half * 256, (half + 1) * 256
                            # ~18-bit approximate reciprocal, ~5x faster than
                            # InstReciprocal; denominators are O(1e1..1e3) so
                            # no edge cases
                            nc.vector.reciprocal_approx_fast(
                                srf[:, h0:h1], sums[:, h0:h1]
                            )
                        return recip

                    def make_recip_cast(srf=srf_p, srec=srec_p):
                        def cast():
                            with nc.allow_low_precision(
                                reason="denominators are O(1e2); bf16 is plenty"
                            ):
                                nc.vector.tensor_copy(srec[:], srf[:])
                        return cast

                    def make_rb_norm(pair2=pair, qb=qb, qsl=qsl, srec=srec_p):
                        # K=1 broadcast matmul + in-place AT multiply for
                        # one head pair; the rb tiles live only within
                        # this piece.
                        def rb_norm():
                            heads2 = (2 * pair2, 2 * pair2 + 1)
                            rbs = {}
                            rb_grp = []
                            for h in heads2:
                                r32 = (h % 2) * 32
                                rb = ps.tile(
                                    [128, 512], f32, tag="aux", bufs=2,
                                    name=f"rb{h}{qb}",
                                )
                                rb_grp.append(nc.tensor.matmul(
                                    rb[0:64, :],
                                    ones_r[r32 : r32 + 1, :],
                                    srec[r32 : r32 + 1, :],
                                    start=True, stop=True,
                                ))
                                rbs[h] = rb
                            pe_group(rb_grp)
                            for h in heads2:
                                base = 64 * (h % 2)
                                nc.vector.tensor_tensor(
                                    AT[base : base + 64, pair2, qsl],
                                    AT[base : base + 64, pair2, qsl],
                                    rbs[h][0:64, :],
                                    MULT,
                                )
                        return rb_norm

                    pair_fin = [make_recip(0), make_recip(1),
                                make_recip_cast(), make_rb_norm()]
                    after_registry[(qb, pair)] = (
                        lambda ps_=pair_fin: fin_pieces.extend(ps_)
                    )
                    if pair == 3:
                        last_pair_fin = pair_fin

                def make_po_half(sc, jh, og_holder, qb=qb):
                    def po_half():
                        if og_holder[0] is None:
                            og_holder[0] = pp.tile(
                                [128, D], bf16, tag="og", bufs=3, name=f"og{sc}"
                            )
                        og = og_holder[0]
                        po = ps.tile(
                            [128, 512], f32, tag="aux", bufs=2,
                            name=f"po{sc}{jh}",
                        )
                        po_grp = []
                        for io in range(4):
                            po_grp.append(nc.tensor.matmul(
                                po[:],
                                AT[:, io, sc * 128 : (sc + 1) * 128],
                                wo[:, io, jh * 512 : (jh + 1) * 512],
                                start=(io == 0), stop=(io == 3),
                            ))
                        pe_group(po_grp)
                        if qb == 3:
                            # qb3's po runs after the last exp — the idle
                            # ACT engine does the copy, off the loaded DVE
                            nc.scalar.copy(
                                og[:, jh * 512 : (jh + 1) * 512], po[:]
                            )
                        else:
                            nc.vector.tensor_copy(
                                og[:, jh * 512 : (jh + 1) * 512], po[:]
                            )
                        if jh == 1:
                            nc.sync.dma_start(
                                out_d[sc * 128 : (sc + 1) * 128, :], og[:]
                            )
                    return po_half

                if qb < 3:
                    qb_fin = []
                    srf_ab = (
                        pp.tile([128, 512], f32, tag="srf", bufs=2, name=f"srfa{qb}"),
                        pp.tile([128, 512], f32, tag="srf", bufs=2, name=f"srfb{qb}"),
                    )

                    def make_recip_ab(idx, half, sums_ab=sums_ab, srf_ab=srf_ab):
                        def recip():
                            h0, h1 = half * 256, (half + 1) * 256
                            nc.vector.reciprocal_approx_fast(
                                srf_ab[idx][:, h0:h1],
                                sums_ab[idx][:, h0:h1],
                            )
                        return recip

                    def make_recip_cast_ab(idx, sums_ab=sums_ab, srf_ab=srf_ab):
                        def cast():
                            with nc.allow_low_precision(
                                reason="denominators are O(1e2); bf16 is plenty"
                            ):
                                nc.vector.tensor_copy(
                                    sums_ab[2 + idx][:], srf_ab[idx][:]
                                )
                        return cast

                    def make_rb_norm_ab(pair2, qb=qb, qsl=qsl, sums_ab=sums_ab):
                        def rb_norm():
                            heads2 = (2 * pair2, 2 * pair2 + 1)
                            p32 = pair2 * 32
                            rbs = {}
                            rb_grp = []
                            for h in heads2:
                                srec = sums_ab[2 + (h % 2)]
                                rb = ps.tile(
                                    [128, 512], f32, tag="aux", bufs=2,
                                    name=f"rb{h}{qb}",
                                )
                                rb_grp.append(nc.tensor.matmul(
                                    rb[0:64, :],
                                    ones_r[p32 : p32 + 1, :],
                                    srec[p32 : p32 + 1, :],
                                    start=True, stop=True,
                                    tile_position=(p32, 0) if p32 == 96 else None,
                                ))
                                rbs[h] = rb
                            pe_group(rb_grp)
                            for h in heads2:
                                base = 64 * (h % 2)
                                nc.vector.tensor_tensor(
                                    AT[base : base + 64, pair2, qsl],
                                    AT[base : base + 64, pair2, qsl],
                                    rbs[h][0:64, :],
                                    MULT,
                                )
                        return rb_norm

                    qb_fin.append(make_recip_ab(0, 0))
                    qb_fin.append(make_recip_ab(0, 1))
                    qb_fin.append(make_recip_cast_ab(0))
                    qb_fin.append(make_recip_ab(1, 0))
                    qb_fin.append(make_recip_ab(1, 1))
                    qb_fin.append(make_recip_cast_ab(1))
                    for pair2 in range(4):
                        qb_fin.append(make_rb_norm_ab(pair2))
                else:
                    # for the last q-block the po pieces ride on pair 3's
                    # callback list (mutated before that flush executes)
                    qb_fin = last_pair_fin
                qb_po = []
                for sc in range(4 * qb, 4 * qb + 4):
                    og_holder = [None]
                    ph = [make_po_half(sc, 0, og_holder),
                          make_po_half(sc, 1, og_holder)]
                    if qb < 2:
                        deferred_po.extend(ph)
                    else:
                        qb_po.extend(ph)
                if qb == 2:
                    # po(qb2) becomes ready filler only once qb2's AT
                    # normalization pieces have been EMITTED (the sentinel
                    # runs after them in the fin FIFO) — emitting po before
                    # the norm would read stale AT.
                    qb_fin.append(lambda ps_=qb_po: proj_q.extend(ps_))
                else:
                    qb_fin.extend(qb_po)
                if qb < 3:
                    after_registry[(qb, 3)] = (
                        lambda ps_=qb_fin: fin_pieces.extend(ps_)
                    )

            flush_pend()
            while fin_pieces or proj_q:
                pump_fin()
                pump_proj(1)

    nc.compile()
    _BUILD_CACHE["nc"] = nc
    return nc


def _host_inputs(x, WQ, WK, WV, WO):
    bf = ml_dtypes.bfloat16
    ki = np.arange(128, dtype=np.float32)[:, None]
    qj = np.arange(128, dtype=np.float32)[None, :]
    # 0/1 stripe mask (multiplied into et after exp): within the partial
    # 128-col stripe of diagonal chunk d, allowed iff local column >= ki.
    # Duplicated side-by-side so one op covers both heads of a pair.
    mask1 = (qj >= ki).astype(np.float32)
    mask = np.concatenate([mask1, mask1], axis=1).astype(bf)
    onesb = np.ones((128, 128), dtype=bf)
    ones32 = np.ones((128, 64), dtype=bf)

    in_maps = []
    for b in range(B):
        xT = np.ascontiguousarray(x[b].T).astype(bf)
        for g in range(G):
            sl = slice(g * DG, (g + 1) * DG)
            in_maps.append(
                {
                    "xT": xT,
                    "wqT": np.ascontiguousarray(WQ[sl, :].T).astype(bf),
                    "wkT": np.ascontiguousarray(WK[sl, :].T).astype(bf),
                    "wvT": np.ascontiguousarray(WV[sl, :].T).astype(bf),
                    "woT": np.ascontiguousarray(WO[:, sl].T).astype(bf),
                    "mask": mask,
                    "onesb": onesb,
                    "ones32": ones32,
                }
            )
    return in_maps


def kernel(x, WQ, WK, WV, WO):
    from concourse.bass_utils import run_bass_kernel_spmd

    x = np.asarray(x, dtype=np.float32)
    WQ = np.asarray(WQ, dtype=np.float32)
    WK = np.asarray(WK, dtype=np.float32)
    WV = np.asarray(WV, dtype=np.float32)
    WO = np.asarray(WO, dtype=np.float32)

    nc = _build()
    in_maps = _host_inputs(x, WQ, WK, WV, WO)
    res = run_bass_kernel_spmd(
        nc,
        in_maps,
        core_ids=list(range(8)),
        trace=bool(os.environ.get("KERNEL_TRACE")),
    )
    kernel.last_results = res
    parts = [r["out"].astype(np.float32) for r in res.results]
    out = np.stack([parts[2 * b] + parts[2 * b + 1] for b in range(B)], axis=0)
    return out.astype(np.float32)
